# revision 12
# baseline (speedup 1.0000x reference)
"""BertSelfAttention with relative_key_query position bias — Trainium2 Bass kernel.

Problem (hardcoded): B=16, S=512, D=1024, 20 heads x 64 (16 base + 4 domain),
dist_emb [1023, 64].  8 NeuronCores, data-parallel over batch (2 per core).

Per-core algorithm (all matmuls on PE, f32r / bf16):
  XT = X^T [1024 din, 1024 tok]  (host-transposed slice of hidden_states)
  QT,KT = (Wcat^T @ XT + b) as [1280 feat, 1024 tok]  (bf16, feat-tiles = head pairs)
  V     = (X @ Wvcat + b)   as [1024 tok, 1280 feat]  (f32)
  per (b, head):
    windowed QE/KE matmuls vs dist-emb tables (bf16), PSUM->SBUF->DRAM scratch,
    shear extraction via skewed DRAM read APs (row-stride 639 on a 640-wide
    layout) gives bias1[l,r] and bias2T[r,l] without any gather ops.
    scoresT[r,l] psum = K^T Q (row-tiled head pairs) + PE-transpose-accumulated
    bias1 blocks; DVE adds bias2T, ACT computes exp(x/8) (no max subtraction —
    logits are O(5)); PV matmul with a ones column (col-tiled at M=64) yields
    ctxT and the softmax denominator in one PSUM; DVE normalizes, PE transposes
    ctx back to [tok, 64].
"""

import os
import tempfile

import numpy as np
import ml_dtypes

import concourse.bass as bass
import concourse.mybir as mybir
import concourse.tile as tile
from concourse import bacc
from concourse import bass_utils
from concourse.masks import make_identity
from contextlib import ExitStack

F32 = mybir.dt.float32
F32R = mybir.dt.float32r
BF16 = mybir.dt.bfloat16

D = 1024          # model dim
F = 1280          # total head features (20 * 64)
S = 512           # sequence length
T = 1024          # tokens per core (2 batches)
H = 20
HD = 64
NPAIR = 10        # head pairs (f-tiles of 128)
NB = 2            # batches per core
WIN = 640         # per-tile window width for QE/KE (639 used + 1 pad)
NCORES = 8

_cache = {}


def _attention_body(nc, tc, ctx, tensors, pools):
    (xt, wq, wk, wv, bq, bk, bv, et2, etr2, y) = tensors

    singles = pools["singles"]
    # --- constant tiles ---
    id128_bf = singles.tile([128, 128], BF16, tag="idbf")
    make_identity(nc, id128_bf)
    id128_f = singles.tile([128, 128], F32, tag="idf")
    make_identity(nc, id128_f)
    ones_f = singles.tile([128, 64], F32, tag="onesf")
    nc.vector.memset(ones_f, 1.0)
    ones_mat = singles.tile([128, 64], F32R, tag="ones")
    nc.scalar.activation(ones_mat, ones_f, func=mybir.ActivationFunctionType.Copy)

    # bias tiles: per-partition layout for Q/K evac ([128, NPAIR]: f = pp*128+p)
    bqc = singles.tile([128, NPAIR], F32, tag="bqc")
    bkc = singles.tile([128, NPAIR], F32, tag="bkc")
    for bt, bsrc in ((bqc, bq), (bkc, bk)):
        nc.sync.dma_start(out=bt, in_=bass.AP(tensor=bsrc, offset=0, ap=[[1, 128], [128, NPAIR]]))
    # broadcast bv across partitions for V evac ([128, F])
    bvb = singles.tile([128, F], F32, tag="bvb")
    nc.sync.dma_start(out=bvb, in_=bass.AP(tensor=bv, offset=0, ap=[[0, 128], [1, F]]))

    # distance-embedding tables, two stacked copies for head-pair row tiling
    ett = singles.tile([128, 1024], BF16, tag="ett")
    etrt = singles.tile([128, 1024], BF16, tag="etrt")
    nc.sync.dma_start(out=ett, in_=et2.ap())
    nc.sync.dma_start(out=etrt, in_=etr2.ap())

    qkv = pools["qkv"]
    qt_sb = qkv.tile([128, NPAIR, T], BF16, tag="qt")         # 2.6MB
    kt_sb = qkv.tile([128, NPAIR, T], BF16, tag="kt")         # 2.6MB
    v_sb = qkv.tile([128, 8, F], F32R, tag="v")                # [p, tt, f] 5.2MB

    mm_ps = pools["mm_ps"]

    # ---------------- projections (xt / w pools scoped to this phase) -----
    with tc.tile_pool(name="xtp", bufs=1) as xt_pool, \
         tc.tile_pool(name="wp", bufs=3) as wpool:
        xt_sb = xt_pool.tile([128, 8, 1024], BF16, tag="xt")   # [p, kt, tok] 4MB
        nc.sync.dma_start(
            out=xt_sb,
            in_=bass.AP(tensor=xt, offset=0, ap=[[1024, 128], [128 * 1024, 8], [1, 1024]]),
        )

        # QT / KT:  out[f, t] = W^T @ XT
        for (w_dram, bias_t, dst) in ((wq, bqc, qt_sb), (wk, bkc, kt_sb)):
            for pp in range(NPAIR):
                wts = []
                for kt in range(8):
                    wt = wpool.tile([128, 128], BF16, tag="wqk", name=f"w_{pp}_{kt}")
                    nc.sync.dma_start(
                        out=wt,
                        in_=bass.AP(tensor=w_dram, offset=kt * 128 * F + pp * 128,
                                    ap=[[F, 128], [1, 128]]),
                    )
                    wts.append(wt)
                for tc_i in range(2):
                    ps = mm_ps.tile([128, 512], F32, tag="ps", bufs=2)
                    for kt in range(8):
                        nc.tensor.matmul(
                            ps,
                            lhsT=wts[kt],
                            rhs=xt_sb[:, kt, tc_i * 512:(tc_i + 1) * 512],
                            start=(kt == 0), stop=(kt == 7),
                        )
                    nc.vector.tensor_scalar_add(
                        dst[:, pp, tc_i * 512:(tc_i + 1) * 512], ps, bias_t[:, pp:pp + 1]
                    )

        # V:  out[t, f] = XT^T @ W
        FCH = (512, 512, 256)
        for tt in range(8):
            for fc in range(3):
                f0 = 512 * fc
                fw = FCH[fc]
                ps = mm_ps.tile([128, 512], F32, tag="ps", bufs=2)
                for kt in range(8):
                    wt = wpool.tile([128, 512], BF16, tag="wv", name=f"wv_{tt}_{fc}_{kt}")
                    nc.sync.dma_start(
                        out=wt[:, 0:fw],
                        in_=bass.AP(tensor=wv, offset=kt * 128 * F + f0,
                                    ap=[[F, 128], [1, fw]]),
                    )
                    nc.tensor.matmul(
                        ps[:, 0:fw],
                        lhsT=xt_sb[:, kt, tt * 128:(tt + 1) * 128],
                        rhs=wt[:, 0:fw],
                        start=(kt == 0), stop=(kt == 7),
                    )
                nc.vector.tensor_add(
                    v_sb[:, tt, f0:f0 + fw], ps[:, 0:fw], bvb[:, f0:f0 + fw]
                )

    # ---------------- attention ----------------
    qe_sb_pool = pools["qe_sb"]
    dram_pool = pools["scratch"]
    b_sb_pool = pools["b_sb"]
    pt_pool = pools["pt"]
    tmp_pool = pools["tmp"]
    ctx_pool = pools["ctx"]
    out_pool = pools["out"]
    sc_ps = pools["mm_ps"]
    b1_ps = pools["mm_ps"]
    pv_ps = pools["mm_ps"]
    ct_ps = pools["mm_ps"]

    for b in range(NB):
        t0 = b * S
        out_sb = out_pool.tile([128, 4, F], F32, tag="out")
        for pp in range(NPAIR):
            # ---- QE / KE windowed matmuls + scratch round trip ----
            # heads: A = partitions 0:64, B = 64:128 of this f-tile
            scr = {}
            for (which, table, qk_t) in (("q", etrt, qt_sb), ("k", ett, kt_sb)):
                ev = {}
                for hh in range(2):
                    ev[hh] = qe_sb_pool.tile([128, 4, WIN], BF16, tag="qe", name=f"qe_{which}_{hh}")
                for lt in range(4):
                    l0 = lt * 128
                    w0 = 384 - l0
                    for hh in range(2):
                        p0 = hh * 64
                        lhsT = qk_t[p0:p0 + 64, pp, t0 + l0:t0 + l0 + 128]
                        ps_a = mm_ps.tile([128, 512], F32, tag="ps", bufs=2)
                        ps_b = mm_ps.tile([128, 128], F32, tag="psb", bufs=1)
                        nc.tensor.matmul(
                            ps_a, lhsT=lhsT, rhs=table[p0:p0 + 64, w0:w0 + 512],
                            start=True, stop=True, tile_position=(p0, 0),
                        )
                        nc.tensor.matmul(
                            ps_b, lhsT=lhsT, rhs=table[p0:p0 + 64, w0 + 512:w0 + 640],
                            start=True, stop=True, tile_position=(p0, 0),
                        )
                        nc.scalar.activation(ev[hh][:, lt, 0:512], ps_a, func=mybir.ActivationFunctionType.Copy)
                        nc.scalar.activation(ev[hh][:, lt, 512:640], ps_b, func=mybir.ActivationFunctionType.Copy)
                for hh in range(2):
                    dt = dram_pool.tile([4, 128, WIN], BF16, tag="scr", name=f"scr_{which}_{hh}")
                    nc.sync.dma_start(
                        out=bass.AP(tensor=dt.tensor, offset=dt.offset,
                                    ap=[[WIN, 128], [128 * WIN, 4], [1, WIN]]),
                        in_=ev[hh],
                    )
                    scr[(which, hh)] = dt

            # ---- shear extraction reads ----
            b1 = {}
            b2t = {}
            for hh in range(2):
                b1[hh] = b_sb_pool.tile([128, 4, 512], BF16, tag="b1", name=f"b1_{hh}")
                nc.sync.dma_start(
                    out=b1[hh],
                    in_=bass.AP(tensor=scr[("q", hh)].tensor,
                                offset=scr[("q", hh)].offset + 127,
                                ap=[[WIN - 1, 128], [128 * WIN, 4], [1, 512]]),
                )
                b2t[hh] = b_sb_pool.tile([128, 4, 512], BF16, tag="b2t", name=f"b2t_{hh}")
                nc.sync.dma_start(
                    out=b2t[hh],
                    in_=bass.AP(tensor=scr[("k", hh)].tensor,
                                offset=scr[("k", hh)].offset + 127,
                                ap=[[WIN - 1, 128], [128 * WIN, 4], [1, 512]]),
                )

            # ---- scoresT = K^T Q + bias1^T + bias2T ; PT = exp(/8) ----
            pt = {}
            for hh in range(2):
                pt[hh] = pt_pool.tile([128, 4, 512], F32R, tag="pt", name=f"pt_{hh}")
            for rt in range(4):
                r0 = rt * 128
                for hh in range(2):
                    p0 = hh * 64
                    ps_s = sc_ps.tile([128, 512], F32, tag="ss", bufs=2)
                    nc.tensor.matmul(
                        ps_s,
                        lhsT=kt_sb[p0:p0 + 64, pp, t0 + r0:t0 + r0 + 128],
                        rhs=qt_sb[p0:p0 + 64, pp, t0:t0 + S],
                        start=True, stop=True, tile_position=(p0, 0),
                    )
                    ps_b1 = b1_ps.tile([128, 512], BF16, tag="sb1", bufs=1)
                    for lc in range(4):
                        nc.tensor.matmul(
                            ps_b1[:, lc * 128:(lc + 1) * 128],
                            lhsT=b1[hh][:, lc, r0:r0 + 128],
                            rhs=id128_bf,
                            is_transpose=True,
                            start=True, stop=True,
                        )
                    t1 = tmp_pool.tile([128, 512], F32, tag="t1")
                    nc.vector.tensor_add(t1, ps_b1, b2t[hh][:, rt, :])
                    tmp = tmp_pool.tile([128, 512], F32, tag="tmp")
                    nc.vector.tensor_add(tmp, ps_s, t1)
                    nc.scalar.activation(
                        pt[hh][:, rt, :], tmp,
                        func=mybir.ActivationFunctionType.Exp, scale=0.125,
                    )

            # ---- PV with ones column; normalize; transpose out ----
            for hh in range(2):
                h = pp * 2 + hh
                ps_pv = pv_ps.tile([64, 512], F32, tag="pv", bufs=1)
                ps_den = pv_ps.tile([64, 512], F32, tag="ctden", bufs=1)
                for rt in range(4):
                    vt_idx = b * 4 + rt
                    nc.tensor.matmul(
                        ps_pv,
                        lhsT=v_sb[:, vt_idx, h * 64:(h + 1) * 64],
                        rhs=pt[hh][:, rt, :],
                        start=(rt == 0), stop=(rt == 3),
                    )
                    nc.tensor.matmul(
                        ps_den,
                        lhsT=ones_mat,
                        rhs=pt[hh][:, rt, :],
                        start=(rt == 0), stop=(rt == 3),
                    )
                pv_sb = ctx_pool.tile([128, 512], F32, tag="pvsb")
                nc.vector.tensor_copy(pv_sb[0:64, :], ps_pv)
                nc.scalar.activation(pv_sb[64:128, :], ps_den,
                                     func=mybir.ActivationFunctionType.Copy)
                for lc in range(4):
                    ps_ct = ct_ps.tile([128, 128], F32, tag="ctden", bufs=1)
                    nc.tensor.matmul(
                        ps_ct,
                        lhsT=pv_sb[:, lc * 128:(lc + 1) * 128],
                        rhs=id128_f,
                        is_transpose=True, start=True, stop=True,
                    )
                    rec = ctx_pool.tile([128, 1], F32, tag="rec")
                    nc.vector.reciprocal(rec, ps_ct[:, 64:65])
                    nc.vector.tensor_scalar_mul(
                        out_sb[:, lc, h * 64:(h + 1) * 64], ps_ct[:, 0:64], rec)

        # write this batch's output [512, 1280]
        nc.sync.dma_start(
            out=bass.AP(tensor=y, offset=b * S * F,
                        ap=[[F, 128], [128 * F, 4], [1, F]]),
            in_=out_sb,
        )


def build(reps: int = 1):
    nc = bacc.Bacc("TRN2", target_bir_lowering=False, debug=False,
                   enable_asserts=False, num_devices=1)
    xt = nc.dram_tensor("xt", [D, T], BF16, kind="ExternalInput")
    wq = nc.dram_tensor("wq", [D, F], BF16, kind="ExternalInput")
    wk = nc.dram_tensor("wk", [D, F], BF16, kind="ExternalInput")
    wv = nc.dram_tensor("wv", [D, F], BF16, kind="ExternalInput")
    bq = nc.dram_tensor("bq", [F], F32, kind="ExternalInput")
    bk = nc.dram_tensor("bk", [F], F32, kind="ExternalInput")
    bv = nc.dram_tensor("bv", [F], F32, kind="ExternalInput")
    et2 = nc.dram_tensor("et2", [128, 1024], BF16, kind="ExternalInput")
    etr2 = nc.dram_tensor("etr2", [128, 1024], BF16, kind="ExternalInput")
    y = nc.dram_tensor("y", [T, F], F32, kind="ExternalOutput")
    tensors = (xt, wq, wk, wv, bq, bk, bv, et2, etr2, y)

    with tile.TileContext(nc) as tc:
        with ExitStack() as ctx:
            pools = {
                "singles": ctx.enter_context(tc.tile_pool(name="singles", bufs=1)),
                "qkv": ctx.enter_context(tc.tile_pool(name="qkv", bufs=1)),
                "out": ctx.enter_context(tc.tile_pool(name="out", bufs=1)),
                "qe_sb": ctx.enter_context(tc.tile_pool(name="qe_sb", bufs=4)),
                "b_sb": ctx.enter_context(tc.tile_pool(name="b_sb", bufs=2)),
                "pt": ctx.enter_context(tc.tile_pool(name="pt", bufs=2)),
                "tmp": ctx.enter_context(tc.tile_pool(name="tmp", bufs=4)),
                "ctx": ctx.enter_context(tc.tile_pool(name="ctx", bufs=2)),
                "scratch": ctx.enter_context(
                    tc.tile_pool(name="scratch", bufs=8, space="DRAM")),
                "mm_ps": ctx.enter_context(
                    tc.tile_pool(name="mm_ps", bufs=1, space="PSUM")),
            }
            if reps == 1:
                _attention_body(nc, tc, ctx, tensors, pools)
            else:
                with tc.For_i(0, reps, 1):
                    _attention_body(nc, tc, ctx, tensors, pools)
    nc.compile()
    return nc


def _prep_inputs(inputs):
    hs = np.asarray(inputs["hidden_states"], np.float32)          # [16, 512, 1024]
    Wq = np.concatenate([np.asarray(inputs["Wq"]), np.asarray(inputs["Wq_dom"])], axis=1)
    Wk = np.concatenate([np.asarray(inputs["Wk"]), np.asarray(inputs["Wk_dom"])], axis=1)
    Wv = np.concatenate([np.asarray(inputs["Wv"]), np.asarray(inputs["Wv_dom"])], axis=1)
    bq = np.concatenate([np.asarray(inputs["bq"]), np.asarray(inputs["bq_dom"])]).astype(np.float32)
    bk = np.concatenate([np.asarray(inputs["bk"]), np.asarray(inputs["bk_dom"])]).astype(np.float32)
    bv = np.concatenate([np.asarray(inputs["bv"]), np.asarray(inputs["bv_dom"])]).astype(np.float32)
    E = np.asarray(inputs["dist_emb"], np.float32)                # [1023, 64]
    ET = E.T                                                      # [64, 1023]
    ET_rev = ET[:, ::-1]
    Wqb = np.ascontiguousarray(Wq).astype(ml_dtypes.bfloat16)
    Wkb = np.ascontiguousarray(Wk).astype(ml_dtypes.bfloat16)
    Wvb = np.ascontiguousarray(Wv).astype(ml_dtypes.bfloat16)
    pad = np.zeros((64, 1), np.float32)
    et1 = np.concatenate([ET, pad], axis=1)                       # [64, 1024]
    etr1 = np.concatenate([ET_rev, pad], axis=1)
    et2 = np.concatenate([et1, et1], axis=0).astype(ml_dtypes.bfloat16)    # [128, 1024]
    etr2 = np.concatenate([etr1, etr1], axis=0).astype(ml_dtypes.bfloat16)

    in_maps = []
    for core in range(NCORES):
        xb = hs[core * NB:(core + 1) * NB].reshape(NB * S, D)
        XT = np.ascontiguousarray(xb.T).astype(ml_dtypes.bfloat16)
        in_maps.append({
            "xt": XT, "wq": Wqb, "wk": Wkb, "wv": Wvb, "bq": bq, "bk": bk, "bv": bv,
            "et2": et2, "etr2": etr2,
        })
    return in_maps


def run(inputs, reps: int = 1):
    key = ("nc", reps)
    if key not in _cache:
        _cache[key] = build(reps)
    nc = _cache[key]
    in_maps = _prep_inputs(inputs)
    res = bass_utils.run_bass_kernel_spmd(nc, in_maps, core_ids=list(range(NCORES)))
    outs = [res.results[i]["y"].reshape(NB, S, F) for i in range(NCORES)]
    return np.concatenate(outs, axis=0)


def kernel(**inputs) -> np.ndarray:
    return run(inputs, reps=1)


# revision 20
# speedup vs baseline: 9.4847x; 9.4847x over previous
"""BertSelfAttention with relative_key_query position bias — Trainium2 Bass kernel.

Problem (hardcoded): B=16, S=512, D=1024, 20 heads x 64 (16 base + 4 domain),
dist_emb [1023, 64].  8 NeuronCores, data-parallel over batch (2 per core).

Per-core algorithm (matmuls in bf16 with f32 PSUM accumulation):
  XT = X^T [1024 din, 1024 tok]  (host-transposed bf16 slice of hidden_states)
  QT,KT = (Wcat^T @ XT + b) as [1280 feat, 1024 tok]  (bf16; feat-tile = head pair)
  V     = (X @ Wvcat + b)   as [1024 tok, 1280 feat]  (bf16)
  per (b, head-pair), heads row-tiled at K=64 in the PE array:
    windowed QE/KE matmuls vs dist-emb tables, PSUM->SBUF(bf16)->DRAM scratch;
    shear extraction via skewed DRAM read APs (row-stride 639 over a 640-wide
    layout) yields bias1[l,r] (read back as f32 via SWDGE cast) and bias2T[r,l]
    (bf16) with plain strided DMAs — no gather ops.
    scoresT[r,l] psum accumulates K^T Q and the PE-transposed bias1 blocks
    (f32 transpose-matmuls); DVE adds bias2T; ACT computes exp(x/8) into bf16
    PT (no max subtraction — logits are O(5)).
    PV matmul (bf16) gives ctxT [64 dh, 512 l]; a parallel ones-matmul gives
    the softmax denominator; both are transposed back via PE so the
    denominator becomes a per-partition scalar for DVE tensor_scalar division.
"""

import numpy as np
import ml_dtypes

import concourse.bass as bass
import concourse.mybir as mybir
import concourse.tile as tile
from concourse import bacc
from concourse import bass_utils
from concourse.masks import make_identity
from contextlib import ExitStack

F32 = mybir.dt.float32
BF16 = mybir.dt.bfloat16

D = 1024          # model dim
F = 1280          # total head features (20 * 64)
S = 512           # sequence length
T = 1024          # tokens per core (2 batches)
NPAIR = 10        # head pairs (f-tiles of 128)
NB = 2            # batches per core
WIN = 640         # per-tile window width for QE/KE (639 used + 1 pad)
NCORES = 8

_cache = {}


def _body(nc, tc, tensors, pools):
    (xt, wq, wk, wv, bq, bk, bv, et2, etr2, y) = tensors

    singles = pools["singles"]
    id128_bf = singles.tile([128, 128], BF16, tag="idbf")
    make_identity(nc, id128_bf)
    id128_f = singles.tile([128, 128], F32, tag="idf")
    make_identity(nc, id128_f)


    # per-partition bias layout for Q/K evac ([128, NPAIR]: f = pp*128 + p)
    bqc = singles.tile([128, NPAIR], F32, tag="bqc")
    bkc = singles.tile([128, NPAIR], F32, tag="bkc")
    for bt, bsrc in ((bqc, bq), (bkc, bk)):
        nc.sync.dma_start(out=bt, in_=bass.AP(tensor=bsrc, offset=0, ap=[[1, 128], [128, NPAIR]]))
    # bv broadcast across partitions for V evac
    bvb = singles.tile([128, F], F32, tag="bvb")
    nc.sync.dma_start(out=bvb, in_=bass.AP(tensor=bv, offset=0, ap=[[0, 128], [1, F]]))

    # distance-embedding tables, two stacked copies for head-pair row tiling
    ett = singles.tile([128, 1024], BF16, tag="ett")
    etrt = singles.tile([128, 1024], BF16, tag="etrt")
    nc.sync.dma_start(out=ett, in_=et2.ap())
    nc.sync.dma_start(out=etrt, in_=etr2.ap())

    qkv = pools["qkv"]
    qt_sb = qkv.tile([128, NPAIR, T], BF16, tag="qt")         # 2.6MB
    kt_sb = qkv.tile([128, NPAIR, T], BF16, tag="kt")         # 2.6MB
    v_sb = qkv.tile([128, 8, 1300], BF16, tag="v")            # [p, tt, f+ones] 2.7MB

    # ones columns interleaved into v_sb (for the PV softmax denominator)
    ones_view = bass.AP(tensor=v_sb.tensor, offset=v_sb.offset + 64,
                        ap=[[8 * 1300, 128], [1300, 8], [65, 20]])
    nc.vector.memset(ones_view, 1.0)

    # ---------------- projections (xt / w pools scoped to this phase) -----
    with tc.tile_pool(name="xtp", bufs=1) as xt_pool, \
         tc.tile_pool(name="wp", bufs=1) as wpool, \
         tc.tile_pool(name="proj_ps", bufs=1, space="PSUM") as proj_ps:
        xt_sb = xt_pool.tile([128, 8, 1024], BF16, tag="xt")  # [p, kt, tok] 2MB
        nc.sync.dma_start(
            out=xt_sb,
            in_=bass.AP(tensor=xt, offset=0, ap=[[1024, 128], [128 * 1024, 8], [1, 1024]]),
        )

        # QT / KT:  out[f, t] = W^T @ XT.  One weight tile per kt, reused for
        # both token chunks; two PSUM banks accumulate in parallel.
        for (w_dram, bias_t, dst) in ((wq, bqc, qt_sb), (wk, bkc, kt_sb)):
            for pp in range(NPAIR):
                ps0 = proj_ps.tile([128, 512], F32, tag="ps", bufs=4, name="ps0")
                ps1 = proj_ps.tile([128, 512], F32, tag="ps", bufs=4, name="ps1")
                wt = wpool.tile([128, 8, 128], BF16, tag="wqk", bufs=3,
                                name=f"w_{pp}")
                nc.sync.dma_start(
                    out=wt,
                    in_=bass.AP(tensor=w_dram, offset=pp * 128,
                                ap=[[F, 128], [128 * F, 8], [1, 128]]),
                )
                for kt in range(8):
                    for tc_i, psd in ((0, ps0), (1, ps1)):
                        nc.tensor.matmul(
                            psd,
                            lhsT=wt[:, kt, :],
                            rhs=xt_sb[:, kt, tc_i * 512:(tc_i + 1) * 512],
                            start=(kt == 0), stop=(kt == 7),
                        )
                for tc_i, psd in ((0, ps0), (1, ps1)):
                    nc.vector.tensor_scalar_add(
                        dst[:, pp, tc_i * 512:(tc_i + 1) * 512], psd,
                        bias_t[:, pp:pp + 1])

        # V:  out[t, f] = XT^T @ W.  Cache the 8 kt weight tiles per f-chunk,
        # loop token tiles inside so each weight loads once.
        FCH = (512, 512, 256)
        for fc in range(3):
            f0 = 512 * fc
            fw = FCH[fc]
            wvt = wpool.tile([128, 8, 512], BF16, tag="wv", bufs=2, name=f"wv_{fc}")
            nc.sync.dma_start(
                out=wvt[:, :, 0:fw],
                in_=bass.AP(tensor=wv, offset=f0,
                            ap=[[F, 128], [128 * F, 8], [1, fw]]),
            )
            for tt in range(8):
                ps = proj_ps.tile([128, 512], F32, tag="ps", bufs=4)
                for kt in range(8):
                    nc.tensor.matmul(
                        ps[:, 0:fw],
                        lhsT=xt_sb[:, kt, tt * 128:(tt + 1) * 128],
                        rhs=wvt[:, kt, 0:fw],
                        start=(kt == 0), stop=(kt == 7),
                    )
                h0 = f0 // 64
                nhead = fw // 64
                vdst = bass.AP(tensor=v_sb.tensor,
                               offset=v_sb.offset + tt * 1300 + h0 * 65,
                               ap=[[8 * 1300, 128], [65, nhead], [1, 64]])
                nc.vector.tensor_add(
                    vdst, ps[:, 0:fw].rearrange("p (h c) -> p h c", c=64),
                    bvb[:, f0:f0 + fw].rearrange("p (h c) -> p h c", c=64)
                )

    # ---------------- attention ----------------
    with ExitStack() as actx:
        qe_sb_pool = actx.enter_context(tc.tile_pool(name="qe_sb", bufs=6))
        attn_ps = actx.enter_context(tc.tile_pool(name="attn_ps", bufs=1, space="PSUM"))
        dram_pool = actx.enter_context(tc.tile_pool(name="scratch", bufs=12, space="DRAM"))
        b_sb_pool = actx.enter_context(tc.tile_pool(name="b_sb", bufs=2))
        pt_pool = actx.enter_context(tc.tile_pool(name="pt", bufs=4))
        tmp_pool = actx.enter_context(tc.tile_pool(name="tmp", bufs=4))
        ctx_pool = actx.enter_context(tc.tile_pool(name="ctx", bufs=3))
        out_pool = pools["out"]
        sc_ps = attn_ps
        pv_ps = attn_ps
        ct_ps = attn_ps

        for b in range(NB):
            t0 = b * S
            out_sb = out_pool.tile([128, 4, F], F32, tag="out")
            for pp in range(NPAIR):
                # ---- QE / KE windowed matmuls + scratch round trip ----
                # heads: A = partitions 0:64, B = 64:128 of this f-tile
                scr = {}
                for (which, table, qk_t) in (("q", etrt, qt_sb), ("k", ett, kt_sb)):
                    ev = {}
                    for hh in range(2):
                        ev[hh] = qe_sb_pool.tile([128, 4, WIN], BF16, tag="qe",
                                                 name=f"qe_{which}_{hh}")
                    for lt in range(4):
                        l0 = lt * 128
                        w0 = 384 - l0
                        for hh in range(2):
                            p0 = hh * 64
                            lhsT = qk_t[p0:p0 + 64, pp, t0 + l0:t0 + l0 + 128]
                            ps_q = attn_ps.tile([128, 1024], F32, tag="qe2", bufs=2,
                                                name="ps_q")
                            nc.tensor.matmul(
                                ps_q[:, 0:512], lhsT=lhsT,
                                rhs=table[p0:p0 + 64, w0:w0 + 512],
                                start=True, stop=True, tile_position=(p0, 0),
                                skip_group_check=True,
                            )
                            nc.tensor.matmul(
                                ps_q[:, 512:640], lhsT=lhsT,
                                rhs=table[p0:p0 + 64, w0 + 512:w0 + 640],
                                start=True, stop=True, tile_position=(p0, 0),
                                skip_group_check=True,
                            )
                            # split evacuation between ACT (head A) and DVE (head B)
                            if hh == 0:
                                nc.scalar.activation(
                                    ev[hh][:, lt, :], ps_q[:, 0:WIN],
                                    func=mybir.ActivationFunctionType.Copy)
                            else:
                                nc.vector.tensor_copy(ev[hh][:, lt, :], ps_q[:, 0:WIN])
                    for hh in range(2):
                        dt = dram_pool.tile([4, 128, WIN], BF16, tag="scr",
                                            name=f"scr_{which}_{hh}")
                        nc.scalar.dma_start(
                            out=bass.AP(tensor=dt.tensor, offset=dt.offset,
                                        ap=[[WIN, 128], [128 * WIN, 4], [1, WIN]]),
                            in_=ev[hh],
                        )
                        scr[(which, hh)] = dt

                # ---- shear extraction reads ----
                # b1 via SWDGE cast to f32 (feeds f32 PE transpose); b2T bf16
                b1 = {}
                b2t = {}
                for hh in range(2):
                    b1[hh] = b_sb_pool.tile([128, 4, 512], BF16, tag="b1",
                                            name=f"b1_{hh}")
                    nc.gpsimd.dma_start(
                        out=b1[hh],
                        in_=bass.AP(tensor=scr[("q", hh)].tensor,
                                    offset=scr[("q", hh)].offset + 127,
                                    ap=[[WIN - 1, 128], [128 * WIN, 4], [1, 512]]),
                    )
                    b2t[hh] = b_sb_pool.tile([128, 4, 512], BF16, tag="b2t",
                                             name=f"b2t_{hh}")
                    nc.sync.dma_start(
                        out=b2t[hh],
                        in_=bass.AP(tensor=scr[("k", hh)].tensor,
                                    offset=scr[("k", hh)].offset + 127,
                                    ap=[[WIN - 1, 128], [128 * WIN, 4], [1, 512]]),
                    )

                # ---- scoresT = K^T Q + bias1^T + bias2T ; PT = exp(/8) ----
                pt = {}
                for hh in range(2):
                    pt[hh] = pt_pool.tile([128, 4, 512], BF16, tag="pt",
                                          name=f"pt_{hh}")
                for rt in range(4):
                    r0 = rt * 128
                    for hh in range(2):
                        p0 = hh * 64
                        ps_s = sc_ps.tile([128, 512], F32, tag="ss", bufs=2)
                        nc.tensor.matmul(
                            ps_s,
                            lhsT=kt_sb[p0:p0 + 64, pp, t0 + r0:t0 + r0 + 128],
                            rhs=qt_sb[p0:p0 + 64, pp, t0:t0 + S],
                            start=True, stop=False, tile_position=(p0, 0),
                            skip_group_check=True,
                        )
                        for lc in range(4):
                            # out[r, l] += sum_k b1[k, r] * id[k, l] = bias1[l, r]
                            nc.tensor.matmul(
                                ps_s[:, lc * 128:(lc + 1) * 128],
                                lhsT=b1[hh][:, lc, r0:r0 + 128],
                                rhs=id128_bf,
                                start=False, stop=(lc == 3),
                                skip_group_check=True,
                            )
                        tmp = tmp_pool.tile([128, 512], F32, tag="tmp")
                        nc.vector.tensor_add(tmp, ps_s, b2t[hh][:, rt, :])
                        nc.scalar.activation(
                            pt[hh][:, rt, :], tmp,
                            func=mybir.ActivationFunctionType.Exp, scale=0.125,
                        )

                # ---- PV + ones-denominator; transpose back; normalize ----
                for hh in range(2):
                    h = pp * 2 + hh
                    ps_pv = pv_ps.tile([65, 512], F32, tag="pv", bufs=1)
                    for rt in range(4):
                        vt_idx = b * 4 + rt
                        nc.tensor.matmul(
                            ps_pv,
                            lhsT=v_sb[:, vt_idx, h * 65:h * 65 + 65],
                            rhs=pt[hh][:, rt, :],
                            start=(rt == 0), stop=(rt == 3),
                        )
                    pv_sb = ctx_pool.tile([65, 512], F32, tag="pvsb")
                    nc.scalar.activation(pv_sb, ps_pv,
                                         func=mybir.ActivationFunctionType.Copy)
                    for lc in range(4):
                        ps_ct = ct_ps.tile([128, 65], F32, tag="ct", bufs=1,
                                           name="ps_ct")
                        nc.tensor.matmul(
                            ps_ct,
                            lhsT=pv_sb[:, lc * 128:(lc + 1) * 128],
                            rhs=id128_f[0:65, 0:65],
                            is_transpose=True, start=True, stop=True,
                        )
                        rec = ctx_pool.tile([128, 1], F32, tag="rec")
                        nc.vector.reciprocal(rec, ps_ct[:, 64:65])
                        nc.vector.tensor_scalar_mul(
                            out_sb[:, lc, h * 64:(h + 1) * 64], ps_ct[:, 0:64], rec)

            # write this batch's output [512, 1280]
            nc.scalar.dma_start(
                out=bass.AP(tensor=y, offset=b * S * F,
                            ap=[[F, 128], [128 * F, 4], [1, F]]),
                in_=out_sb,
            )


def build(reps: int = 1):
    nc = bacc.Bacc("TRN2", target_bir_lowering=False, debug=False,
                   enable_asserts=False, num_devices=1)
    xt = nc.dram_tensor("xt", [D, T], BF16, kind="ExternalInput")
    wq = nc.dram_tensor("wq", [D, F], BF16, kind="ExternalInput")
    wk = nc.dram_tensor("wk", [D, F], BF16, kind="ExternalInput")
    wv = nc.dram_tensor("wv", [D, F], BF16, kind="ExternalInput")
    bq = nc.dram_tensor("bq", [F], F32, kind="ExternalInput")
    bk = nc.dram_tensor("bk", [F], F32, kind="ExternalInput")
    bv = nc.dram_tensor("bv", [F], F32, kind="ExternalInput")
    et2 = nc.dram_tensor("et2", [128, 1024], BF16, kind="ExternalInput")
    etr2 = nc.dram_tensor("etr2", [128, 1024], BF16, kind="ExternalInput")
    y = nc.dram_tensor("y", [T, F], F32, kind="ExternalOutput")
    tensors = (xt, wq, wk, wv, bq, bk, bv, et2, etr2, y)

    with tile.TileContext(nc) as tc:
        with ExitStack() as ctx:
            pools = {
                "singles": ctx.enter_context(tc.tile_pool(name="singles", bufs=1)),
                "qkv": ctx.enter_context(tc.tile_pool(name="qkv", bufs=1)),
                "out": ctx.enter_context(tc.tile_pool(name="out", bufs=1)),
            }
            if reps == 1:
                _body(nc, tc, tensors, pools)
            else:
                with tc.For_i(0, reps, 1):
                    _body(nc, tc, tensors, pools)
    nc.compile()
    return nc


def _prep_inputs(inputs):
    hs = np.asarray(inputs["hidden_states"], np.float32)          # [16, 512, 1024]
    Wq = np.concatenate([np.asarray(inputs["Wq"]), np.asarray(inputs["Wq_dom"])], axis=1)
    Wk = np.concatenate([np.asarray(inputs["Wk"]), np.asarray(inputs["Wk_dom"])], axis=1)
    Wv = np.concatenate([np.asarray(inputs["Wv"]), np.asarray(inputs["Wv_dom"])], axis=1)
    bq = np.concatenate([np.asarray(inputs["bq"]), np.asarray(inputs["bq_dom"])]).astype(np.float32)
    bk = np.concatenate([np.asarray(inputs["bk"]), np.asarray(inputs["bk_dom"])]).astype(np.float32)
    bv = np.concatenate([np.asarray(inputs["bv"]), np.asarray(inputs["bv_dom"])]).astype(np.float32)
    E = np.asarray(inputs["dist_emb"], np.float32)                # [1023, 64]
    ET = E.T                                                      # [64, 1023]
    ET_rev = ET[:, ::-1]
    Wqb = np.ascontiguousarray(Wq).astype(ml_dtypes.bfloat16)
    Wkb = np.ascontiguousarray(Wk).astype(ml_dtypes.bfloat16)
    Wvb = np.ascontiguousarray(Wv).astype(ml_dtypes.bfloat16)
    pad = np.zeros((64, 1), np.float32)
    et1 = np.concatenate([ET, pad], axis=1)                       # [64, 1024]
    etr1 = np.concatenate([ET_rev, pad], axis=1)
    et2 = np.concatenate([et1, et1], axis=0).astype(ml_dtypes.bfloat16)    # [128, 1024]
    etr2 = np.concatenate([etr1, etr1], axis=0).astype(ml_dtypes.bfloat16)

    in_maps = []
    for core in range(NCORES):
        xb = hs[core * NB:(core + 1) * NB].reshape(NB * S, D)
        XT = np.ascontiguousarray(xb.T).astype(ml_dtypes.bfloat16)
        in_maps.append({
            "xt": XT, "wq": Wqb, "wk": Wkb, "wv": Wvb, "bq": bq, "bk": bk, "bv": bv,
            "et2": et2, "etr2": etr2,
        })
    return in_maps


def run(inputs, reps: int = 1):
    key = ("nc", reps)
    if key not in _cache:
        _cache[key] = build(reps)
    nc = _cache[key]
    in_maps = _prep_inputs(inputs)
    res = bass_utils.run_bass_kernel_spmd(nc, in_maps, core_ids=list(range(NCORES)))
    outs = [res.results[i]["y"].reshape(NB, S, F) for i in range(NCORES)]
    return np.concatenate(outs, axis=0)


def kernel(**inputs) -> np.ndarray:
    return run(inputs, reps=1)
